# revision 38
# baseline (speedup 1.0000x reference)
"""Trainium2 Bass kernel for nn_AttentionA (dual-direction attention block).

Math (per reference):
  xn = LayerNorm(x);  q = rope(heads(xn@Wq));  k,v = heads(split(xn@Wkv))
  S[b,h,i,j] = q_i . k_j          (no 1/sqrt(d) scale)
  dir1: out1 = merge(softmax_j(S) @ v) @ Wout
  dir2: aw = S^T, P2 = softmax_i(aw);  xa_up[i,d] = (sum_j P2[j,i]) * v[i,d]
        out2 = merge(xa_up) @ Wout

Key facts exploited:
  * max|S| ~= 51  ->  exp() without max-subtraction is fp32-safe, so ONE
    E = exp(S) feeds both softmax directions.
  * dir2 collapses to a column-sum reweighting of v:
        s2[i] = sum_j E[i,j] / c[j],  c[j] = sum_i E[i,j]
  * E is computed in transposed layout E^T[j, i]; the PV matmul
    lhsT = [v | ones | 1/c] gives x_up^T, r[i] and s2[i] in one pass.
  * LayerNorm is folded into the projection epilogue:
        proj^T = A . g  +  cvec (x) h  + bias,   A = W'^T x^T,
        W' = diag(ln_w) W, cvec = W'^T 1, h = -mu*g, bias = ln_b @ W.

Sharding: 16 heads / 8 cores = 2 heads per core (tensor parallel); every
core sees full x; per-core output partials (128 of 1024 contraction rows
of Wout) are summed on the host.
"""

import os
import numpy as np

B, N, D, H, HD = 2, 2048, 1024, 16, 64
R = B * N            # 4096 flattened rows
P = 128              # partitions
KT = D // P          # 8 k-tiles over the model dim
CH = 512             # projection row-chunk
NCH = R // CH        # 8 chunks
NCORES = 8
EPS = 1e-5

_NC_CACHE = {}
TRACE = False
LAST_RESULTS = None


def _rope_tables():
    """cos / signed-sin tables in q^T layout [128, N].  The fp32 phases reach
    ~1e6 rad, so they must be produced by the *same* fp32 ops (XLA on CPU) as
    the reference — numpy's linspace/pow differ by ~17 ULP, which decorrelates
    the high-frequency entries entirely."""
    import jax
    import jax.numpy as jnp
    try:
        cpu = jax.devices("cpu")[0]
        ctx = jax.default_device(cpu)
    except Exception:
        import contextlib
        ctx = contextlib.nullcontext()
    with ctx:
        top = 2595.0 * jnp.log10(jnp.asarray(1.0 + 4000.0 / 200.0,
                                             dtype=jnp.float32))
        lin = jnp.linspace(0.0, top, HD // 2, dtype=jnp.float32)
        mel = jnp.power(10.0, lin / 2595.0) - 1.0
        base = 200.0 * mel / 1000.0
        freqs = (36000.0 / 220.0) * base                       # [32] fp32
        pos = jnp.arange(N, dtype=jnp.float32)
        f = pos[:, None] * freqs[None, :]                      # [N, 32] fp32
        f = jnp.repeat(f, 2, axis=-1)                          # [N, 64]
        cos = np.asarray(jnp.cos(f), np.float32)
        sin = np.asarray(jnp.sin(f), np.float32)
    sign = np.where(np.arange(HD) % 2 == 0, np.float32(-1.0), np.float32(1.0))
    sins = sin * sign[None, :]
    cos_t = np.ascontiguousarray(np.concatenate([cos.T, cos.T], axis=0))   # [128, N]
    sin_t = np.ascontiguousarray(np.concatenate([sins.T, sins.T], axis=0))
    return cos_t, sin_t


def _build_nc():
    import concourse.bass as bass
    import concourse.bacc as bacc
    import concourse.mybir as mybir
    import concourse.tile as tile

    fp32 = mybir.dt.float32
    f32r = mybir.dt.float32r
    bf16 = mybir.dt.bfloat16
    AF = mybir.ActivationFunctionType
    OP = mybir.AluOpType

    nc = bacc.Bacc()
    xT = nc.dram_tensor("xT", [D, R], f32r, kind="ExternalInput")
    wq = nc.dram_tensor("wq", [D, P], fp32, kind="ExternalInput")
    wk = nc.dram_tensor("wk", [D, P], fp32, kind="ExternalInput")
    wv = nc.dram_tensor("wv", [D, P], fp32, kind="ExternalInput")
    wo = nc.dram_tensor("wo", [P, D], fp32, kind="ExternalInput")
    lnw = nc.dram_tensor("lnw", [P, KT], fp32, kind="ExternalInput")
    lnb = nc.dram_tensor("lnb", [P, KT], fp32, kind="ExternalInput")
    cost = nc.dram_tensor("cost", [P, N], fp32, kind="ExternalInput")
    sint = nc.dram_tensor("sint", [P, N], fp32, kind="ExternalInput")
    ident = nc.dram_tensor("ident", [P, P], fp32, kind="ExternalInput")
    o1 = nc.dram_tensor("o1", [R, D], bf16, kind="ExternalOutput")
    o2 = nc.dram_tensor("o2", [R, D], bf16, kind="ExternalOutput")

    xT_r = xT[:].rearrange("(o p) r -> p o r", p=P)
    shuf_mask = [p ^ 1 for p in range(32)]

    with tile.TileContext(nc) as tc:
        with tc.tile_pool(name="glob", bufs=1) as glob:
            qT = glob.tile([P, R], f32r, tag="qT")
            kTt = glob.tile([P, R], f32r, tag="kTt")
            vTt = glob.tile([P, R], fp32, tag="vTt")
            id_s = glob.tile([P, P], fp32, tag="id")
            invd_f = glob.tile([P, 1], fp32, tag="invd_f")
            invd = glob.tile([P, 1], f32r, tag="invd")
            onecol = glob.tile([P, 1], fp32, tag="onecol")
            onecol_r = glob.tile([P, 1], bf16, tag="onecol_r")
            zero_r = glob.tile([P, 1], bf16, tag="zero_r")
            epst = glob.tile([P, 1], fp32, tag="epst")
            zerot = glob.tile([P, 1], fp32, tag="zerot")
            negone = glob.tile([P, 1], fp32, tag="negone")
            nc.sync.dma_start(id_s[:], ident[:])
            nc.vector.memset(invd_f[:], 1.0 / D)
            nc.vector.tensor_copy(invd[:], invd_f[:])
            nc.vector.memset(onecol[:], 1.0)
            nc.vector.tensor_copy(onecol_r[:], onecol[:])
            nc.vector.memset(epst[:], EPS)
            nc.vector.memset(zerot[:], 0.0)
            nc.vector.memset(negone[:], -1.0)
            nc.vector.tensor_copy(zero_r[:], zerot[:])

            # ---------------- Phase B: LN-folded projections -------------
            with tc.tile_pool(name="pbw", bufs=1) as pbw, \
                 tc.tile_pool(name="pbx", bufs=3) as pbx, \
                 tc.tile_pool(name="pbsq", bufs=1) as pbsq, \
                 tc.tile_pool(name="pbs", bufs=2) as pbs, \
                 tc.tile_pool(name="pstat", bufs=2, space="PSUM") as pstat, \
                 tc.tile_pool(name="pproj", bufs=3, space="PSUM") as pproj:

                xc0 = pbx.tile([P, KT, CH], f32r, tag="xc")
                for kt in range(KT):
                    nc.scalar.dma_start(xc0[:, kt, :], xT_r[:, kt, 0:CH])
                w_s = {}
                for nm, dram in (("q", wq), ("k", wk), ("v", wv)):
                    t = pbw.tile([P, KT, P], fp32, tag=f"w{nm}")
                    dr = dram[:].rearrange("(o p) m -> p o m", p=P)
                    for kt in range(KT):
                        nc.sync.dma_start(t[:, kt, :], dr[:, kt, :])
                    w_s[nm] = t
                lnw_s = pbw.tile([P, KT], fp32, tag="lnw")
                lnb_s = pbw.tile([P, KT], fp32, tag="lnb")
                cos_s = pbw.tile([P, N], fp32, tag="cos")
                sin_s = pbw.tile([P, N], fp32, tag="sin")
                nc.sync.dma_start(lnw_s[:], lnw[:])
                nc.sync.dma_start(lnb_s[:], lnb[:])
                for q4 in range(4):
                    nc.sync.dma_start(cos_s[:, q4 * 512:(q4 + 1) * 512],
                                      cost[:, q4 * 512:(q4 + 1) * 512])
                    nc.sync.dma_start(sin_s[:, q4 * 512:(q4 + 1) * 512],
                                      sint[:, q4 * 512:(q4 + 1) * 512])

                # bias vectors (raw W), then fold ln_w, then colsum vectors
                bias = {}
                for nm in ("q", "k", "v"):
                    ps = pproj.tile([P, CH], fp32, tag="a")
                    for kt in range(KT):
                        nc.tensor.matmul(ps[:, 0:1], w_s[nm][:, kt, :],
                                         lnb_s[:, kt:kt + 1],
                                         start=(kt == 0), stop=(kt == KT - 1))
                    bv = pbw.tile([P, 1], fp32, tag=f"b{nm}")
                    nc.vector.tensor_copy(bv[:], ps[:, 0:1])
                    bias[nm] = bv
                for nm in ("q", "k", "v"):
                    for kt in range(KT):
                        nc.vector.tensor_scalar_mul(
                            w_s[nm][:, kt, :], w_s[nm][:, kt, :], lnw_s[:, kt:kt + 1])
                csum = {}
                for nm in ("q", "k", "v"):
                    ps = pproj.tile([P, CH], fp32, tag="a")
                    for kt in range(KT):
                        nc.tensor.matmul(ps[:, 0:1], w_s[nm][:, kt, :], onecol[:],
                                         start=(kt == 0), stop=(kt == KT - 1))
                    cv = pbw.tile([P, 1], fp32, tag=f"c{nm}")
                    nc.vector.tensor_copy(cv[:], ps[:, 0:1])
                    csum[nm] = cv
                w_r = {}
                for nm in ("q", "k", "v"):
                    t = pbw.tile([P, KT, P], f32r, tag=f"wr{nm}")
                    for kt in range(KT):
                        nc.vector.tensor_copy(t[:, kt, :], w_s[nm][:, kt, :])
                    w_r[nm] = t

                for ci in range(NCH):
                    rs = ci * CH
                    if ci == 0:
                        xc = xc0
                    else:
                        xc = pbx.tile([P, KT, CH], f32r, tag="xc")
                        for kt in range(KT):
                            nc.scalar.dma_start(xc[:, kt, :], xT_r[:, kt, rs:rs + CH])

                    ps_mu = pstat.tile([1, CH], fp32, tag="mu")
                    for kt in range(KT):
                        nc.tensor.matmul(ps_mu[:], invd[:], xc[:, kt, :],
                                         start=(kt == 0), stop=(kt == KT - 1))
                    xsq = pbsq.tile([P, KT, CH], f32r, tag="xsq")
                    for kt in range(KT):
                        eng = nc.vector if kt < 4 else nc.gpsimd
                        eng.tensor_mul(xsq[:, kt, :], xc[:, kt, :].bitcast(fp32),
                                       xc[:, kt, :].bitcast(fp32))
                    ps_sq = pstat.tile([1, CH], fp32, tag="sq")
                    for kt in range(KT):
                        nc.tensor.matmul(ps_sq[:], invd[:], xsq[:, kt, :],
                                         start=(kt == 0), stop=(kt == KT - 1))

                    mu_sb = pbs.tile([1, CH], fp32, tag="mu_sb")
                    nc.vector.tensor_copy(mu_sb[:], ps_mu[:])
                    negmu2 = pbs.tile([1, CH], fp32, tag="negmu2")
                    nc.vector.scalar_tensor_tensor(
                        negmu2[:], mu_sb[:], negone[0:1, :], mu_sb[:],
                        OP.mult, OP.mult)
                    var = pbs.tile([1, CH], fp32, tag="var")
                    nc.vector.tensor_add(var[:], ps_sq[:], negmu2[:])
                    sdv = pbs.tile([1, CH], fp32, tag="sdv")
                    nc.scalar.activation(sdv[:], var[:], AF.Sqrt, bias=epst[0:1, :])
                    g = pbs.tile([1, CH], fp32, tag="g")
                    nc.vector.reciprocal(g[:], sdv[:])
                    h = pbs.tile([1, CH], fp32, tag="h")
                    nc.vector.scalar_tensor_tensor(
                        h[:], mu_sb[:], negone[0:1, :], g[:], OP.mult, OP.mult)
                    Gb = pbs.tile([P, CH], fp32, tag="Gb")
                    nc.gpsimd.partition_broadcast(Gb[:], g[:])
                    Hb = pbs.tile([P, CH], fp32, tag="Hb")
                    nc.gpsimd.partition_broadcast(Hb[:], h[:])

                    for nm, dest in (("q", qT), ("k", kTt), ("v", vTt)):
                        ps_a = pproj.tile([P, CH], fp32, tag="a")
                        for kt in range(KT):
                            nc.tensor.matmul(ps_a[:], w_r[nm][:, kt, :], xc[:, kt, :],
                                             start=(kt == 0), stop=(kt == KT - 1))
                        dch = dest[:, rs:rs + CH]
                        dchf = dch.bitcast(fp32)
                        nc.vector.tensor_mul(dch, ps_a[:], Gb[:])
                        nc.vector.scalar_tensor_tensor(
                            dch, Hb[:], csum[nm][:], dchf, OP.mult, OP.add)
                        nc.vector.tensor_scalar_add(dch, dchf, bias[nm][:])

                    # rope(q) in-place on this chunk
                    ns = rs % N
                    qch = qT[:, rs:rs + CH]
                    qchf = qch.bitcast(fp32)
                    shuf = pbs.tile([P, CH], fp32, tag="shuf")
                    nc.vector.stream_shuffle(shuf[:], qchf, shuf_mask)
                    nc.gpsimd.tensor_mul(shuf[:], shuf[:], sin_s[:, ns:ns + CH])
                    nc.vector.tensor_mul(qch, qchf, cos_s[:, ns:ns + CH])
                    nc.vector.tensor_add(qch, qchf, shuf[:])

            # ---------------- Phase C: attention + output ----------------
            with tc.tile_pool(name="pcg", bufs=1) as pcg, \
                 tc.tile_pool(name="pcE", bufs=4) as pcE, \
                 tc.tile_pool(name="pcs", bufs=4) as pcs, \
                 tc.tile_pool(name="pcb", bufs=2) as pcb, \
                 tc.tile_pool(name="pva", bufs=1) as pva, \
                 tc.tile_pool(name="sbout", bufs=4) as sbout, \
                 tc.tile_pool(name="psqk", bufs=2, space="PSUM") as psqk, \
                 tc.tile_pool(name="pspv", bufs=2, space="PSUM") as pspv:
                xupm = pcg.tile([P, R], bf16, tag="xupm")
                vtld = pcg.tile([P, R], bf16, tag="vtld")
                xuph = [pcg.tile([64, R], bf16, tag=f"xuph{hh}",
                                 name=f"xuph{hh}") for hh in range(2)]
                wo_s = pcg.tile([P, D], fp32, tag="wo")
                nc.sync.dma_start(wo_s[:], wo[:])
                wo_r = pcg.tile([P, D], bf16, tag="wor")
                nc.vector.tensor_copy(wo_r[:, 0:512], wo_s[:, 0:512])
                nc.vector.tensor_copy(wo_r[:, 512:1024], wo_s[:, 512:1024])

                # v in natural layout + v_aug assembly, both batches up front
                vaug = [pva.tile([P, 16, 97], bf16, tag=f"vaug{vb}",
                                 name=f"vaug{vb}") for vb in range(4)]
                for vb in range(4):
                    nc.gpsimd.tensor_copy(
                        vaug[vb][:, :, 65:96],
                        zero_r[:, :, None].to_broadcast((P, 16, 31)))
                    nc.gpsimd.tensor_copy(
                        vaug[vb][:, :, 64:65],
                        onecol_r[:, :, None].to_broadcast((P, 16, 1)))
                for b in range(B):
                    for J in range(16):
                        tp = psqk.tile([P, P], fp32, tag="qk")
                        nc.tensor.transpose(
                            tp[:], vTt[:, b * N + J * P: b * N + (J + 1) * P],
                            id_s[:])
                        for hh in range(2):
                            nc.vector.tensor_copy(
                                vaug[2 * b + hh][:, J, 0:64],
                                tp[:, hh * 64:(hh + 1) * 64])

                for b in range(B):
                    base = b * N
                    for hh in range(2):
                        hs = hh * 64
                        va = vaug[2 * b + hh]
                        pv = [pspv.tile([97, 1024], fp32, tag="pv",
                                        name=f"pv{ihc}") for ihc in range(2)]
                        for J in range(16):
                            jb = base + J * P
                            E = pcE.tile([P, N], bf16, tag="E")
                            cacc = pcs.tile([P, 2], fp32, tag="cacc")
                            for ih in range(2):
                                psE = psqk.tile([P, 1024], fp32, tag="qk")
                                for q2 in range(2):
                                    io = base + ih * 1024 + q2 * 512
                                    nc.tensor.matmul(
                                        psE[:, q2 * 512:(q2 + 1) * 512],
                                        kTt[hs:hs + 64, jb:jb + P],
                                        qT[hs:hs + 64, io:io + 512],
                                        start=True, stop=True)
                                nc.scalar.activation(
                                    E[:, ih * 1024:(ih + 1) * 1024], psE[:],
                                    AF.Exp, bias=zerot[:],
                                    accum_out=cacc[:, ih:ih + 1])
                            cs = pcs.tile([P, 1], fp32, tag="cs")
                            nc.vector.tensor_add(cs[:], cacc[:, 0:1], cacc[:, 1:2])
                            wtmp = pcs.tile([P, 1], fp32, tag="wtmp")
                            nc.vector.reciprocal(wtmp[:], cs[:])
                            nc.vector.tensor_copy(va[:, J, 96:97], wtmp[:])
                            for ic in range(4):
                                nc.tensor.matmul(
                                    pv[ic // 2][:, (ic % 2) * 512:(ic % 2 + 1) * 512],
                                    va[:, J, :],
                                    E[:, ic * 512:(ic + 1) * 512],
                                    start=(J == 0), stop=(J == 15))

                        def _epilogue(hh, hs, pv, ihc, base=base):
                            cc = base + ihc * 1024
                            # pv rows 64 / 96 hold r (sum E) and s2
                            ex = pcs.tile([P, 1024], fp32, tag="ex", name="ex")
                            nc.vector.tensor_copy(ex[64:97, :], pv[ihc][64:97, :])
                            nc.vector.reciprocal(ex[64:65, :], ex[64:65, :])
                            rr = pcs.tile([1, 1024], fp32, tag="rr", name="rr")
                            nc.sync.dma_start(rr[:], ex[64:65, :])
                            ss = pcs.tile([1, 1024], fp32, tag="ss", name="ss")
                            nc.sync.dma_start(ss[:], ex[96:97, :])
                            rb = pcb.tile([64, 1024], fp32, tag="rb", name="rb")
                            nc.gpsimd.partition_broadcast(rb[:], rr[:])
                            s2b = pcb.tile([P, 1024], fp32, tag="s2b", name="s2b")
                            nc.gpsimd.partition_broadcast(s2b[:], ss[:])
                            nc.vector.tensor_mul(xuph[hh][:, cc:cc + 1024],
                                                 pv[ihc][0:64, :], rb[:])
                            nc.vector.tensor_mul(
                                vtld[hs:hs + 64, cc:cc + 1024],
                                vTt[hs:hs + 64, cc:cc + 1024],
                                s2b[hs:hs + 64, :])

                        def _merge(ihc, base=base):
                            for mh in range(2):
                                nc.sync.dma_start(
                                    xupm[mh * 64:(mh + 1) * 64,
                                         base + ihc * 1024:base + (ihc + 1) * 1024],
                                    xuph[mh][:, base + ihc * 1024:
                                              base + (ihc + 1) * 1024])

                        def _outproj(ib_lo, ib_hi, base=base, b=b):
                            for ib in range(ib_lo, ib_hi):
                                rs = base + ib * P
                                for oi, (srt, dst) in enumerate(((xupm, o1),
                                                                 (vtld, o2))):
                                    ps = psqk.tile([P, 1024], fp32, tag="qk",
                                                   name="ops")
                                    for oc in range(2):
                                        nc.tensor.matmul(
                                            ps[:, oc * 512:(oc + 1) * 512],
                                            srt[:, rs:rs + P],
                                            wo_r[:, oc * 512:(oc + 1) * 512],
                                            start=True, stop=True)
                                    ot = sbout.tile([P, D], bf16, tag="ot",
                                                    name="ot")
                                    if (ib * 2 + oi) % 16 < 7:
                                        nc.scalar.copy(ot[:], ps[:])
                                    else:
                                        nc.vector.tensor_copy(ot[:], ps[:])
                                    nc.sync.dma_start(dst[rs:rs + P, :], ot[:])

                        if hh == 0:
                            _epilogue(0, 0, pv, 0)
                            _epilogue(0, 0, pv, 1)
                            pv0 = pv
                        else:
                            _epilogue(1, 64, pv, 0)
                            _merge(0)
                            _epilogue(1, 64, pv, 1)
                            _outproj(0, 8)
                            _merge(1)
                            _outproj(8, 16)
    nc.compile()
    return nc


def _get_nc():
    if "nc" not in _NC_CACHE:
        _NC_CACHE["nc"] = _build_nc()
    return _NC_CACHE["nc"]


def kernel(x, ln_w, ln_b, Wq, Wkv, Wout):
    global LAST_RESULTS
    from concourse import bass_utils

    x = np.asarray(x, np.float32)
    ln_w = np.asarray(ln_w, np.float32)
    ln_b = np.asarray(ln_b, np.float32)
    Wq = np.asarray(Wq, np.float32)
    Wkv = np.asarray(Wkv, np.float32)
    Wout = np.asarray(Wout, np.float32)

    xT = np.ascontiguousarray(x.reshape(R, D).T)               # [D, R]
    lnw_t = np.ascontiguousarray(ln_w.reshape(KT, P).T)        # [P, KT]
    lnb_t = np.ascontiguousarray(ln_b.reshape(KT, P).T)
    cos_t, sin_t = _rope_tables()
    ident = np.eye(P, dtype=np.float32)

    in_maps = []
    for c in range(NCORES):
        cs = c * P
        in_maps.append({
            "xT": xT,
            "wq": np.ascontiguousarray(Wq[:, cs:cs + P]),
            "wk": np.ascontiguousarray(Wkv[:, cs:cs + P]),
            "wv": np.ascontiguousarray(Wkv[:, D + cs:D + cs + P]),
            "wo": np.ascontiguousarray(Wout[cs:cs + P, :]),
            "lnw": lnw_t,
            "lnb": lnb_t,
            "cost": cos_t,
            "sint": sin_t,
            "ident": ident,
        })

    nc = _get_nc()
    res = bass_utils.run_bass_kernel_spmd(
        nc, in_maps, core_ids=list(range(NCORES)), trace=TRACE)
    LAST_RESULTS = res

    o1 = np.zeros((R, D), np.float32)
    o2 = np.zeros((R, D), np.float32)
    for r in res.results:
        o1 += np.asarray(r["o1"], np.float32)
        o2 += np.asarray(r["o2"], np.float32)
    return o1.reshape(B, N, D), o2.reshape(B, N, D)


# revision 39
# speedup vs baseline: 1.0075x; 1.0075x over previous
"""Trainium2 Bass kernel for nn_AttentionA (dual-direction attention block).

Math (per reference):
  xn = LayerNorm(x);  q = rope(heads(xn@Wq));  k,v = heads(split(xn@Wkv))
  S[b,h,i,j] = q_i . k_j          (no 1/sqrt(d) scale)
  dir1: out1 = merge(softmax_j(S) @ v) @ Wout
  dir2: aw = S^T, P2 = softmax_i(aw);  xa_up[i,d] = (sum_j P2[j,i]) * v[i,d]
        out2 = merge(xa_up) @ Wout

Key facts exploited:
  * max|S| ~= 51  ->  exp() without max-subtraction is fp32-safe, so ONE
    E = exp(S) feeds both softmax directions.
  * dir2 collapses to a column-sum reweighting of v:
        s2[i] = sum_j E[i,j] / c[j],  c[j] = sum_i E[i,j]
  * E is computed in transposed layout E^T[j, i]; the PV matmul
    lhsT = [v | ones | 1/c] gives x_up^T, r[i] and s2[i] in one pass.
  * LayerNorm is folded into the projection epilogue:
        proj^T = A . g  +  cvec (x) h  + bias,   A = W'^T x^T,
        W' = diag(ln_w) W, cvec = W'^T 1, h = -mu*g, bias = ln_b @ W.

Sharding: 16 heads / 8 cores = 2 heads per core (tensor parallel); every
core sees full x; per-core output partials (128 of 1024 contraction rows
of Wout) are summed on the host.
"""

import os
import numpy as np

B, N, D, H, HD = 2, 2048, 1024, 16, 64
R = B * N            # 4096 flattened rows
P = 128              # partitions
KT = D // P          # 8 k-tiles over the model dim
CH = 512             # projection row-chunk
NCH = R // CH        # 8 chunks
NCORES = 8
EPS = 1e-5

_NC_CACHE = {}
TRACE = False
LAST_RESULTS = None


def _rope_tables():
    """cos / signed-sin tables in q^T layout [128, N].  The fp32 phases reach
    ~1e6 rad, so they must be produced by the *same* fp32 ops (XLA on CPU) as
    the reference — numpy's linspace/pow differ by ~17 ULP, which decorrelates
    the high-frequency entries entirely."""
    import jax
    import jax.numpy as jnp
    try:
        cpu = jax.devices("cpu")[0]
        ctx = jax.default_device(cpu)
    except Exception:
        import contextlib
        ctx = contextlib.nullcontext()
    with ctx:
        top = 2595.0 * jnp.log10(jnp.asarray(1.0 + 4000.0 / 200.0,
                                             dtype=jnp.float32))
        lin = jnp.linspace(0.0, top, HD // 2, dtype=jnp.float32)
        mel = jnp.power(10.0, lin / 2595.0) - 1.0
        base = 200.0 * mel / 1000.0
        freqs = (36000.0 / 220.0) * base                       # [32] fp32
        pos = jnp.arange(N, dtype=jnp.float32)
        f = pos[:, None] * freqs[None, :]                      # [N, 32] fp32
        f = jnp.repeat(f, 2, axis=-1)                          # [N, 64]
        cos = np.asarray(jnp.cos(f), np.float32)
        sin = np.asarray(jnp.sin(f), np.float32)
    sign = np.where(np.arange(HD) % 2 == 0, np.float32(-1.0), np.float32(1.0))
    sins = sin * sign[None, :]
    cos_t = np.ascontiguousarray(np.concatenate([cos.T, cos.T], axis=0))   # [128, N]
    sin_t = np.ascontiguousarray(np.concatenate([sins.T, sins.T], axis=0))
    return cos_t, sin_t


def _build_nc():
    import concourse.bass as bass
    import concourse.bacc as bacc
    import concourse.mybir as mybir
    import concourse.tile as tile

    fp32 = mybir.dt.float32
    f32r = mybir.dt.float32r
    bf16 = mybir.dt.bfloat16
    AF = mybir.ActivationFunctionType
    OP = mybir.AluOpType

    nc = bacc.Bacc()
    xT = nc.dram_tensor("xT", [D, R], f32r, kind="ExternalInput")
    wq = nc.dram_tensor("wq", [D, P], fp32, kind="ExternalInput")
    wk = nc.dram_tensor("wk", [D, P], fp32, kind="ExternalInput")
    wv = nc.dram_tensor("wv", [D, P], fp32, kind="ExternalInput")
    wo = nc.dram_tensor("wo", [P, D], fp32, kind="ExternalInput")
    lnw = nc.dram_tensor("lnw", [P, KT], fp32, kind="ExternalInput")
    lnb = nc.dram_tensor("lnb", [P, KT], fp32, kind="ExternalInput")
    cost = nc.dram_tensor("cost", [P, N], fp32, kind="ExternalInput")
    sint = nc.dram_tensor("sint", [P, N], fp32, kind="ExternalInput")
    ident = nc.dram_tensor("ident", [P, P], fp32, kind="ExternalInput")
    o1 = nc.dram_tensor("o1", [R, D], bf16, kind="ExternalOutput")
    o2 = nc.dram_tensor("o2", [R, D], bf16, kind="ExternalOutput")

    xT_r = xT[:].rearrange("(o p) r -> p o r", p=P)
    shuf_mask = [p ^ 1 for p in range(32)]

    with tile.TileContext(nc) as tc:
        with tc.tile_pool(name="glob", bufs=1) as glob:
            qT = glob.tile([P, R], f32r, tag="qT")
            kTt = glob.tile([P, R], f32r, tag="kTt")
            vTt = glob.tile([P, R], fp32, tag="vTt")
            id_s = glob.tile([P, P], fp32, tag="id")
            invd_f = glob.tile([P, 1], fp32, tag="invd_f")
            invd = glob.tile([P, 1], f32r, tag="invd")
            onecol = glob.tile([P, 1], fp32, tag="onecol")
            onecol_r = glob.tile([P, 1], bf16, tag="onecol_r")
            zero_r = glob.tile([P, 1], bf16, tag="zero_r")
            epst = glob.tile([P, 1], fp32, tag="epst")
            zerot = glob.tile([P, 1], fp32, tag="zerot")
            negone = glob.tile([P, 1], fp32, tag="negone")
            nc.sync.dma_start(id_s[:], ident[:])
            nc.vector.memset(invd_f[:], 1.0 / D)
            nc.vector.tensor_copy(invd[:], invd_f[:])
            nc.vector.memset(onecol[:], 1.0)
            nc.vector.tensor_copy(onecol_r[:], onecol[:])
            nc.vector.memset(epst[:], EPS)
            nc.vector.memset(zerot[:], 0.0)
            nc.vector.memset(negone[:], -1.0)
            nc.vector.tensor_copy(zero_r[:], zerot[:])

            # ---------------- Phase B: LN-folded projections -------------
            with tc.tile_pool(name="pbw", bufs=1) as pbw, \
                 tc.tile_pool(name="pbx", bufs=3) as pbx, \
                 tc.tile_pool(name="pbsq", bufs=1) as pbsq, \
                 tc.tile_pool(name="pbs", bufs=2) as pbs, \
                 tc.tile_pool(name="pstat", bufs=2, space="PSUM") as pstat, \
                 tc.tile_pool(name="pproj", bufs=3, space="PSUM") as pproj:

                xc0 = pbx.tile([P, KT, CH], f32r, tag="xc")
                for kt in range(KT):
                    nc.scalar.dma_start(xc0[:, kt, :], xT_r[:, kt, 0:CH])
                w_s = {}
                for nm, dram in (("q", wq), ("k", wk), ("v", wv)):
                    t = pbw.tile([P, KT, P], fp32, tag=f"w{nm}")
                    dr = dram[:].rearrange("(o p) m -> p o m", p=P)
                    for kt in range(KT):
                        nc.sync.dma_start(t[:, kt, :], dr[:, kt, :])
                    w_s[nm] = t
                lnw_s = pbw.tile([P, KT], fp32, tag="lnw")
                lnb_s = pbw.tile([P, KT], fp32, tag="lnb")
                cos_s = pbw.tile([P, N], fp32, tag="cos")
                sin_s = pbw.tile([P, N], fp32, tag="sin")
                nc.sync.dma_start(lnw_s[:], lnw[:])
                nc.sync.dma_start(lnb_s[:], lnb[:])
                for q4 in range(4):
                    nc.sync.dma_start(cos_s[:, q4 * 512:(q4 + 1) * 512],
                                      cost[:, q4 * 512:(q4 + 1) * 512])
                    nc.sync.dma_start(sin_s[:, q4 * 512:(q4 + 1) * 512],
                                      sint[:, q4 * 512:(q4 + 1) * 512])

                # bias vectors (raw W), then fold ln_w, then colsum vectors
                bias = {}
                for nm in ("q", "k", "v"):
                    ps = pproj.tile([P, CH], fp32, tag="a")
                    for kt in range(KT):
                        nc.tensor.matmul(ps[:, 0:1], w_s[nm][:, kt, :],
                                         lnb_s[:, kt:kt + 1],
                                         start=(kt == 0), stop=(kt == KT - 1))
                    bv = pbw.tile([P, 1], fp32, tag=f"b{nm}")
                    nc.vector.tensor_copy(bv[:], ps[:, 0:1])
                    bias[nm] = bv
                for nm in ("q", "k", "v"):
                    for kt in range(KT):
                        nc.vector.tensor_scalar_mul(
                            w_s[nm][:, kt, :], w_s[nm][:, kt, :], lnw_s[:, kt:kt + 1])
                csum = {}
                for nm in ("q", "k", "v"):
                    ps = pproj.tile([P, CH], fp32, tag="a")
                    for kt in range(KT):
                        nc.tensor.matmul(ps[:, 0:1], w_s[nm][:, kt, :], onecol[:],
                                         start=(kt == 0), stop=(kt == KT - 1))
                    cv = pbw.tile([P, 1], fp32, tag=f"c{nm}")
                    nc.vector.tensor_copy(cv[:], ps[:, 0:1])
                    csum[nm] = cv
                w_r = {}
                for nm in ("q", "k", "v"):
                    t = pbw.tile([P, KT, P], f32r, tag=f"wr{nm}")
                    for kt in range(KT):
                        nc.vector.tensor_copy(t[:, kt, :], w_s[nm][:, kt, :])
                    w_r[nm] = t

                for ci in range(NCH):
                    rs = ci * CH
                    if ci == 0:
                        xc = xc0
                    else:
                        xc = pbx.tile([P, KT, CH], f32r, tag="xc")
                        for kt in range(KT):
                            nc.scalar.dma_start(xc[:, kt, :], xT_r[:, kt, rs:rs + CH])

                    ps_mu = pstat.tile([1, CH], fp32, tag="mu")
                    for kt in range(KT):
                        nc.tensor.matmul(ps_mu[:], invd[:], xc[:, kt, :],
                                         start=(kt == 0), stop=(kt == KT - 1))
                    xsq = pbsq.tile([P, KT, CH], f32r, tag="xsq")
                    for kt in range(KT):
                        eng = nc.vector if kt < 4 else nc.gpsimd
                        eng.tensor_mul(xsq[:, kt, :], xc[:, kt, :].bitcast(fp32),
                                       xc[:, kt, :].bitcast(fp32))
                    ps_sq = pstat.tile([1, CH], fp32, tag="sq")
                    for kt in range(KT):
                        nc.tensor.matmul(ps_sq[:], invd[:], xsq[:, kt, :],
                                         start=(kt == 0), stop=(kt == KT - 1))

                    mu_sb = pbs.tile([1, CH], fp32, tag="mu_sb")
                    nc.vector.tensor_copy(mu_sb[:], ps_mu[:])
                    negmu2 = pbs.tile([1, CH], fp32, tag="negmu2")
                    nc.vector.scalar_tensor_tensor(
                        negmu2[:], mu_sb[:], negone[0:1, :], mu_sb[:],
                        OP.mult, OP.mult)
                    var = pbs.tile([1, CH], fp32, tag="var")
                    nc.vector.tensor_add(var[:], ps_sq[:], negmu2[:])
                    sdv = pbs.tile([1, CH], fp32, tag="sdv")
                    nc.scalar.activation(sdv[:], var[:], AF.Sqrt, bias=epst[0:1, :])
                    g = pbs.tile([1, CH], fp32, tag="g")
                    nc.vector.reciprocal(g[:], sdv[:])
                    h = pbs.tile([1, CH], fp32, tag="h")
                    nc.vector.scalar_tensor_tensor(
                        h[:], mu_sb[:], negone[0:1, :], g[:], OP.mult, OP.mult)
                    Gb = pbs.tile([P, CH], fp32, tag="Gb")
                    nc.gpsimd.partition_broadcast(Gb[:], g[:])
                    Hb = pbs.tile([P, CH], fp32, tag="Hb")
                    nc.gpsimd.partition_broadcast(Hb[:], h[:])

                    for nm, dest in (("q", qT), ("k", kTt), ("v", vTt)):
                        ps_a = pproj.tile([P, CH], fp32, tag="a")
                        for kt in range(KT):
                            nc.tensor.matmul(ps_a[:], w_r[nm][:, kt, :], xc[:, kt, :],
                                             start=(kt == 0), stop=(kt == KT - 1))
                        dch = dest[:, rs:rs + CH]
                        dchf = dch.bitcast(fp32)
                        nc.vector.tensor_mul(dch, ps_a[:], Gb[:])
                        nc.vector.scalar_tensor_tensor(
                            dch, Hb[:], csum[nm][:], dchf, OP.mult, OP.add)
                        nc.vector.tensor_scalar_add(dch, dchf, bias[nm][:])

                    # rope(q) in-place on this chunk
                    ns = rs % N
                    qch = qT[:, rs:rs + CH]
                    qchf = qch.bitcast(fp32)
                    shuf = pbs.tile([P, CH], fp32, tag="shuf")
                    nc.vector.stream_shuffle(shuf[:], qchf, shuf_mask)
                    nc.gpsimd.tensor_mul(shuf[:], shuf[:], sin_s[:, ns:ns + CH])
                    nc.vector.tensor_mul(qch, qchf, cos_s[:, ns:ns + CH])
                    nc.vector.tensor_add(qch, qchf, shuf[:])

            # ---------------- Phase C: attention + output ----------------
            with tc.tile_pool(name="pcg", bufs=1) as pcg, \
                 tc.tile_pool(name="pcE", bufs=4) as pcE, \
                 tc.tile_pool(name="pcs", bufs=4) as pcs, \
                 tc.tile_pool(name="pcb", bufs=2) as pcb, \
                 tc.tile_pool(name="pva", bufs=1) as pva, \
                 tc.tile_pool(name="sbout", bufs=4) as sbout, \
                 tc.tile_pool(name="psqk", bufs=2, space="PSUM") as psqk, \
                 tc.tile_pool(name="pspv", bufs=2, space="PSUM") as pspv:
                xupm = pcg.tile([P, R], bf16, tag="xupm")
                vtld = pcg.tile([P, R], bf16, tag="vtld")
                xuph = [pcg.tile([64, R], bf16, tag=f"xuph{hh}",
                                 name=f"xuph{hh}") for hh in range(2)]
                wo_s = pcg.tile([P, D], fp32, tag="wo")
                nc.sync.dma_start(wo_s[:], wo[:])
                wo_r = pcg.tile([P, D], bf16, tag="wor")
                nc.vector.tensor_copy(wo_r[:, 0:512], wo_s[:, 0:512])
                nc.vector.tensor_copy(wo_r[:, 512:1024], wo_s[:, 512:1024])

                # v in natural layout + v_aug assembly, both batches up front
                vaug = [pva.tile([P, 16, 97], bf16, tag=f"vaug{vb}",
                                 name=f"vaug{vb}") for vb in range(4)]
                for vb in range(4):
                    nc.gpsimd.tensor_copy(
                        vaug[vb][:, :, 65:96],
                        zero_r[:, :, None].to_broadcast((P, 16, 31)))
                    nc.gpsimd.tensor_copy(
                        vaug[vb][:, :, 64:65],
                        onecol_r[:, :, None].to_broadcast((P, 16, 1)))
                for b in range(B):
                    for J in range(16):
                        tp = psqk.tile([P, P], fp32, tag="qk")
                        nc.tensor.transpose(
                            tp[:], vTt[:, b * N + J * P: b * N + (J + 1) * P],
                            id_s[:])
                        for hh in range(2):
                            nc.vector.tensor_copy(
                                vaug[2 * b + hh][:, J, 0:64],
                                tp[:, hh * 64:(hh + 1) * 64])

                for b in range(B):
                    base = b * N
                    for hh in range(2):
                        hs = hh * 64
                        va = vaug[2 * b + hh]
                        pv = [pspv.tile([97, 1024], fp32, tag="pv",
                                        name=f"pv{ihc}") for ihc in range(2)]
                        for J in range(16):
                            jb = base + J * P
                            E = pcE.tile([P, N], bf16, tag="E")
                            cacc = pcs.tile([P, 2], fp32, tag="cacc")
                            for ih in range(2):
                                psE = psqk.tile([P, 1024], fp32, tag="qk")
                                for q2 in range(2):
                                    io = base + ih * 1024 + q2 * 512
                                    nc.tensor.matmul(
                                        psE[:, q2 * 512:(q2 + 1) * 512],
                                        kTt[hs:hs + 64, jb:jb + P],
                                        qT[hs:hs + 64, io:io + 512],
                                        start=True, stop=True)
                                nc.scalar.activation(
                                    E[:, ih * 1024:(ih + 1) * 1024], psE[:],
                                    AF.Exp, bias=zerot[:],
                                    accum_out=cacc[:, ih:ih + 1])
                            cs = pcs.tile([P, 1], fp32, tag="cs")
                            nc.vector.tensor_add(cs[:], cacc[:, 0:1], cacc[:, 1:2])
                            wtmp = pcs.tile([P, 1], fp32, tag="wtmp")
                            nc.vector.reciprocal(wtmp[:], cs[:])
                            nc.vector.tensor_copy(va[:, J, 96:97], wtmp[:])
                            for ic in range(4):
                                nc.tensor.matmul(
                                    pv[ic // 2][:, (ic % 2) * 512:(ic % 2 + 1) * 512],
                                    va[:, J, :],
                                    E[:, ic * 512:(ic + 1) * 512],
                                    start=(J == 0), stop=(J == 15))

                        def _epilogue(hh, hs, pv, ihc, base=base):
                            cc = base + ihc * 1024
                            # pv rows 64 / 96 hold r (sum E) and s2
                            ex = pcs.tile([P, 1024], fp32, tag="ex", name="ex")
                            nc.vector.tensor_copy(ex[64:97, :], pv[ihc][64:97, :])
                            nc.vector.reciprocal(ex[64:65, :], ex[64:65, :])
                            rr = pcs.tile([1, 1024], fp32, tag="rr", name="rr")
                            nc.sync.dma_start(rr[:], ex[64:65, :])
                            ss = pcs.tile([1, 1024], fp32, tag="ss", name="ss")
                            nc.sync.dma_start(ss[:], ex[96:97, :])
                            rb = pcb.tile([64, 1024], fp32, tag="rb", name="rb")
                            nc.gpsimd.partition_broadcast(rb[:], rr[:])
                            s2b = pcb.tile([P, 1024], fp32, tag="s2b", name="s2b")
                            nc.gpsimd.partition_broadcast(s2b[:], ss[:])
                            nc.vector.tensor_mul(xuph[hh][:, cc:cc + 1024],
                                                 pv[ihc][0:64, :], rb[:])
                            nc.vector.tensor_mul(
                                vtld[hs:hs + 64, cc:cc + 1024],
                                vTt[hs:hs + 64, cc:cc + 1024],
                                s2b[hs:hs + 64, :])

                        def _merge(ihc, base=base):
                            for mh in range(2):
                                nc.sync.dma_start(
                                    xupm[mh * 64:(mh + 1) * 64,
                                         base + ihc * 1024:base + (ihc + 1) * 1024],
                                    xuph[mh][:, base + ihc * 1024:
                                              base + (ihc + 1) * 1024])

                        def _outproj(ib_lo, ib_hi, base=base, b=b):
                            for ib in range(ib_lo, ib_hi):
                                rs = base + ib * P
                                for oi, (srt, dst) in enumerate(((xupm, o1),
                                                                 (vtld, o2))):
                                    ps = psqk.tile([P, 1024], fp32, tag="qk",
                                                   name="ops")
                                    for oc in range(2):
                                        nc.tensor.matmul(
                                            ps[:, oc * 512:(oc + 1) * 512],
                                            srt[:, rs:rs + P],
                                            wo_r[:, oc * 512:(oc + 1) * 512],
                                            start=True, stop=True)
                                    ot = sbout.tile([P, D], bf16, tag="ot",
                                                    name="ot")
                                    if (ib * 2 + oi) % 8 < 5:
                                        nc.scalar.copy(ot[:], ps[:])
                                    else:
                                        nc.vector.tensor_copy(ot[:], ps[:])
                                    nc.sync.dma_start(dst[rs:rs + P, :], ot[:])

                        if hh == 0:
                            _epilogue(0, 0, pv, 0)
                            _epilogue(0, 0, pv, 1)
                            pv0 = pv
                        else:
                            _epilogue(1, 64, pv, 0)
                            _merge(0)
                            _epilogue(1, 64, pv, 1)
                            _outproj(0, 8)
                            _merge(1)
                            _outproj(8, 16)
    nc.compile()
    return nc


def _get_nc():
    if "nc" not in _NC_CACHE:
        _NC_CACHE["nc"] = _build_nc()
    return _NC_CACHE["nc"]


def kernel(x, ln_w, ln_b, Wq, Wkv, Wout):
    global LAST_RESULTS
    from concourse import bass_utils

    x = np.asarray(x, np.float32)
    ln_w = np.asarray(ln_w, np.float32)
    ln_b = np.asarray(ln_b, np.float32)
    Wq = np.asarray(Wq, np.float32)
    Wkv = np.asarray(Wkv, np.float32)
    Wout = np.asarray(Wout, np.float32)

    xT = np.ascontiguousarray(x.reshape(R, D).T)               # [D, R]
    lnw_t = np.ascontiguousarray(ln_w.reshape(KT, P).T)        # [P, KT]
    lnb_t = np.ascontiguousarray(ln_b.reshape(KT, P).T)
    cos_t, sin_t = _rope_tables()
    ident = np.eye(P, dtype=np.float32)

    in_maps = []
    for c in range(NCORES):
        cs = c * P
        in_maps.append({
            "xT": xT,
            "wq": np.ascontiguousarray(Wq[:, cs:cs + P]),
            "wk": np.ascontiguousarray(Wkv[:, cs:cs + P]),
            "wv": np.ascontiguousarray(Wkv[:, D + cs:D + cs + P]),
            "wo": np.ascontiguousarray(Wout[cs:cs + P, :]),
            "lnw": lnw_t,
            "lnb": lnb_t,
            "cost": cos_t,
            "sint": sin_t,
            "ident": ident,
        })

    nc = _get_nc()
    res = bass_utils.run_bass_kernel_spmd(
        nc, in_maps, core_ids=list(range(NCORES)), trace=TRACE)
    LAST_RESULTS = res

    o1 = np.zeros((R, D), np.float32)
    o2 = np.zeros((R, D), np.float32)
    for r in res.results:
        o1 += np.asarray(r["o1"], np.float32)
        o2 += np.asarray(r["o2"], np.float32)
    return o1.reshape(B, N, D), o2.reshape(B, N, D)
